# revision 1
# baseline (speedup 1.0000x reference)
"""ChannelAttention Trainium2 Bass kernel.

Data-parallel over batch: 8 batches -> 8 NeuronCores, zero communication.

Per core (x [4096, 768]):
  pass 1:  transpose x on PE; q,k = x @ Wqk (token-major); accumulate the
           per-head gram matrices q_h^T k_h and the q/k column sums of
           squares in persistent PSUM banks across all 32 token chunks.
  finalize: norms -> scales 1/max(||.||, eps) (temperature folded in),
           softmax of the 96x96 per-head channel attention, then fold the
           whole output path into one matrix:
             Wbig = sum_h Wv_h @ attn_h^T @ Wproj_h   [768, 768]
           (valid because attention mixes channels, not tokens).
  pass 2:  y = x @ Wbig + bias, streamed from the resident x^T.

All fp32 data; matmuls run as float32r (FP22 multiply, fp32 accumulate).
"""

import sys

if "/opt/trn_rl_repo" not in sys.path:
    sys.path.insert(0, "/opt/trn_rl_repo")

import numpy as np

N, C, H, HD = 4096, 768, 8, 96
NC3 = 3 * C
EPS = 1e-12
P = 128
CB = C // P           # 6 contraction chunks

_CACHE = {}


def _build(n_tokens=N):
    import concourse.bacc as bacc
    import concourse.tile as tile
    import concourse.mybir as mybir
    from concourse.masks import make_identity
    from contextlib import ExitStack

    F32 = mybir.dt.float32
    F32R = mybir.dt.float32r

    def R(ap):
        return ap.bitcast(F32R)

    nch = n_tokens // P

    nc = bacc.Bacc("TRN2", target_bir_lowering=False, debug=False, num_devices=8)
    x = nc.dram_tensor("x", [n_tokens, C], F32, kind="ExternalInput")
    wqkv = nc.dram_tensor("wqkv", [C, NC3], F32, kind="ExternalInput")
    temp = nc.dram_tensor("temp", [H], F32, kind="ExternalInput")
    wproj = nc.dram_tensor("wproj", [C, C], F32, kind="ExternalInput")
    bproj = nc.dram_tensor("bproj", [C], F32, kind="ExternalInput")
    y = nc.dram_tensor("y", [n_tokens, C], F32, kind="ExternalOutput")

    with tile.TileContext(nc) as tc, ExitStack() as ctx:
        singles = ctx.enter_context(tc.tile_pool(name="singles", bufs=1))

        # ---- all persistent tiles allocated first so transient pools
        # (wqk, finalize scratch) stack above them and get reclaimed ----
        wv_sb = singles.tile([P, CB, C], F32R)  # v columns of Wqkv
        temp_sb = singles.tile([HD, H], F32)
        ident_f = singles.tile([P, P], F32)
        ident = singles.tile([P, P], F32R)
        ones_f = singles.tile([P, HD], F32)
        ones32 = singles.tile([P, 32], F32R)
        xt_sb = singles.tile([P, CB, n_tokens], F32R)  # resident x^T
        s_sb = singles.tile([HD, 2 * H], F32)  # 1/norm columns, q then k
        sumsq_sb = singles.tile([HD, 2 * H], F32)
        wbig_sb = singles.tile([P, CB, C], F32R)  # fused output matrix

        nc.sync.dma_start(out=temp_sb, in_=temp[None, :].to_broadcast([HD, H]))
        make_identity(nc, ident_f)
        nc.vector.tensor_copy(out=ident, in_=ident_f)
        nc.vector.memset(ones_f, 1.0)
        nc.vector.tensor_copy(out=ones32, in_=ones_f[:, 0:32])

        # x chunk 0 is DMA'd first (inside the chunk loop); wqk lands as
        # six per-cb tiles so qk matmuls start as each piece arrives
        wqk_ctx = ExitStack()
        wqk_pool = wqk_ctx.enter_context(tc.tile_pool(name="wqk", bufs=1))
        wqk_cb = [
            wqk_pool.tile([P, 2 * C], F32R, tag=f"wqk{cb}", name=f"wqk{cb}")
            for cb in range(CB)
        ]

        # ---- pass-1 PSUM: gram attn (2) + sumsq (3) + qk (2) + tp (1) ----
        gram_ctx = ExitStack()
        gram_pool = gram_ctx.enter_context(
            tc.tile_pool(name="gram", bufs=1, space="PSUM")
        )
        gram = [
            gram_pool.tile([P, 512], F32, tag=f"g{i}", name=f"gram{i}")
            for i in range(5)
        ]

        with tc.tile_pool(name="qkps", bufs=2, space="PSUM") as qkpool, \
             tc.tile_pool(name="tpps", bufs=1, space="PSUM") as tppool, \
             tc.tile_pool(name="p1", bufs=2) as p1pool:
            for i in range(nch):
                nsl_i = slice(i * P, (i + 1) * P)
                xc = p1pool.tile([P, C], F32, tag="xc")
                nc.sync.dma_start(out=xc, in_=x[nsl_i, :])
                if i == 0:
                    for cb in range(CB):
                        nc.sync.dma_start(
                            out=wqk_cb[cb],
                            in_=R(wqkv[cb * P : (cb + 1) * P, 0 : 2 * C]),
                        )

                # transpose x chunk -> xt_sb[:, cb, i*128:+128]
                for g in range(2):
                    ntp = 4 if g == 0 else CB - 4
                    tp = tppool.tile([P, 512], F32, tag="tp")
                    for q in range(ntp):
                        cb = g * 4 + q
                        nc.tensor.transpose(
                            tp[:, q * P : (q + 1) * P],
                            xc[:, cb * P : (cb + 1) * P],
                            ident_f,
                        )
                    for q in range(ntp):
                        cb = g * 4 + q
                        nc.scalar.copy(
                            out=xt_sb[:, cb, nsl_i],
                            in_=tp[:, q * P : (q + 1) * P],
                        )

                # q,k matmuls: [128n, 1536]
                qkc = p1pool.tile([P, 2 * C], F32R, tag="qkc")
                for js in range(3):
                    jsl = slice(js * 512, (js + 1) * 512)
                    qkps = qkpool.tile([P, 512], F32, tag="qkps")
                    for cb in range(CB):
                        nc.tensor.matmul(
                            qkps,
                            lhsT=R(xt_sb[:, cb, nsl_i]),
                            rhs=R(wqk_cb[cb][:, jsl]),
                            start=(cb == 0),
                            stop=(cb == CB - 1),
                        )
                    if js == 1:
                        nc.scalar.copy(out=qkc[:, jsl], in_=qkps)
                    else:
                        nc.vector.tensor_copy(out=qkc[:, jsl], in_=qkps)

                sqc = p1pool.tile([P, 2 * C], F32R, tag="sqc")
                nc.vector.tensor_tensor(
                    out=sqc, in0=qkc, in1=qkc, op=mybir.AluOpType.mult
                )

                # gram: attn_raw accumulation (bank0: heads 0-4, bank1: 5-7)
                for h in range(H):
                    bank = gram[0] if h < 5 else gram[1]
                    co = HD * h if h < 5 else HD * (h - 5)
                    nc.tensor.matmul(
                        bank[0:HD, co : co + HD],
                        lhsT=R(qkc[:, h * HD : (h + 1) * HD]),
                        rhs=R(qkc[:, C + h * HD : C + (h + 1) * HD]),
                        start=(i == 0 and h in (0, 5)),
                        stop=(i == nch - 1 and h in (4, 7)),
                        skip_group_check=True,
                    )
                # sum-of-squares: banks 2-4, one [32, 512] matmul each
                for s in range(3):
                    nc.tensor.matmul(
                        gram[2 + s][0:32, 0:512],
                        lhsT=R(ones32),
                        rhs=R(sqc[:, 512 * s : 512 * (s + 1)]),
                        start=(i == 0),
                        stop=(i == nch - 1),
                        skip_group_check=True,
                    )

        wqk_ctx.close()

        # weights for the finalize fusion stream in during late pass 1
        nc.sync.dma_start(
            out=wv_sb,
            in_=R(wqkv[:, 2 * C : NC3].rearrange("(co ci) j -> ci co j", ci=P)),
        )

        # ---------------- finalize ----------------
        with tc.tile_pool(name="fsing", bufs=1) as fsing, \
             tc.tile_pool(name="fsb", bufs=2) as fsb:
            wpe_ctx = ExitStack()
            wpe_pool = wpe_ctx.enter_context(tc.tile_pool(name="wpe", bufs=1))
            wprojE_sb = wpe_pool.tile([HD, H, C], F32R)  # Wproj rows, per head
            nc.sync.dma_start(
                out=wprojE_sb, in_=R(wproj.rearrange("(h d) c -> d h c", h=H))
            )
            gram_sb = fsing.tile([P, 5, 512], F32R)
            for b in range(5):
                nc.vector.tensor_copy(out=gram_sb[:, b, :], in_=gram[b])
            gram_ctx.close()

            t1_sb = fsing.tile([HD, H, C], F32R)  # attn_h^T @ Wproj_h
            skrep_sb = fsing.tile([HD, H, HD], F32)

            with tc.tile_pool(name="fpsB", bufs=1, space="PSUM") as fpB, \
                 tc.tile_pool(name="fskp", bufs=1, space="PSUM") as fskp, \
                 tc.tile_pool(name="fwvt", bufs=1, space="PSUM") as fwvt, \
                 tc.tile_pool(name="ft1", bufs=2, space="PSUM") as ft1:
                # sumsq rows -> columns [96,1] via K=1 fp32 matmuls
                gsf = gram_sb.rearrange("p b c -> p (b c)")
                sqp = fpB.tile([HD, 2 * H], F32, tag="sqp")
                for h in range(H):
                    for t in range(2):
                        f0 = 2 * 512 + t * C + h * HD
                        j = t * H + h
                        nc.tensor.matmul(
                            sqp[:, j : j + 1],
                            lhsT=gsf[0:1, f0 : f0 + HD].bitcast(F32),
                            rhs=ones_f[0:1, 0:1],
                            start=(j == 0),
                            stop=(j == 2 * H - 1),
                            skip_group_check=True,
                        )
                nc.vector.tensor_copy(out=sumsq_sb, in_=sqp)

                # s = 1/max(sqrt(ss), eps); fold temperature into s_q
                nc.scalar.sqrt(out=s_sb, in_=sumsq_sb)
                nc.vector.tensor_scalar_max(s_sb, s_sb, EPS)
                nc.vector.reciprocal(out=s_sb, in_=s_sb)
                nc.vector.tensor_tensor(
                    out=s_sb[:, 0:H],
                    in0=s_sb[:, 0:H],
                    in1=temp_sb,
                    op=mybir.AluOpType.mult,
                )

                # replicate k-scale across rows: ones96.T @ diag_all where
                # diag_all[:, h, :] = diag(s_k_h), built in one batched op
                ones96 = fsb.tile([HD, HD], F32R, tag="ones96")
                nc.vector.tensor_copy(out=ones96, in_=ones_f[0:HD, :])
                diag_all = fsb.tile([HD, H, HD], F32R, tag="diag_all")
                nc.vector.tensor_tensor(
                    out=diag_all,
                    in0=ident[0:HD, None, 0:HD].to_broadcast([HD, H, HD]),
                    in1=s_sb[:, H : 2 * H, None].to_broadcast([HD, H, HD]),
                    op=mybir.AluOpType.mult,
                )
                skp = fskp.tile([HD, H * HD], F32, tag="skp")
                nc.tensor.matmul(
                    skp[:, 0:512], lhsT=R(ones96),
                    rhs=R(diag_all).rearrange("p h e -> p (h e)")[:, 0:512],
                    start=True, stop=True,
                )
                nc.tensor.matmul(
                    skp[:, 512:768], lhsT=R(ones96),
                    rhs=R(diag_all).rearrange("p h e -> p (h e)")[:, 512:768],
                    start=True, stop=True,
                )
                nc.vector.tensor_copy(
                    out=skrep_sb.rearrange("p h e -> p (h e)"), in_=skp
                )

                # batched softmax per bank-group (no max subtraction:
                # |attn| <= |temperature|, exp is safe)
                at_all = fsing.tile([HD, H, HD], F32R)
                for g, (h0, nh) in enumerate(((0, 5), (5, 3))):
                    ga = at_all[:, h0 : h0 + nh, :]
                    nc.vector.tensor_tensor(
                        out=ga,
                        in0=gram_sb[0:HD, g, 0 : nh * HD].rearrange(
                            "p (h e) -> p h e", e=HD
                        ),
                        in1=s_sb[:, h0 : h0 + nh, None].to_broadcast(
                            [HD, nh, HD]
                        ),
                        op=mybir.AluOpType.mult,
                    )
                    nc.vector.tensor_tensor(
                        out=ga, in0=ga, in1=skrep_sb[:, h0 : h0 + nh, :],
                        op=mybir.AluOpType.mult,
                    )
                    nc.scalar.activation(
                        out=ga, in_=ga,
                        func=mybir.ActivationFunctionType.Exp,
                        bias=0.0, scale=1.0,
                    )
                    rsum = fsb.tile([HD, H], F32, tag="rsum")
                    nc.vector.tensor_reduce(
                        out=rsum[:, 0:nh], in_=ga, axis=mybir.AxisListType.X,
                        op=mybir.AluOpType.add,
                    )
                    nc.vector.reciprocal(out=rsum[:, 0:nh], in_=rsum[:, 0:nh])
                    nc.vector.tensor_tensor(
                        out=ga, in0=ga,
                        in1=rsum[:, 0:nh, None].to_broadcast([HD, nh, HD]),
                        op=mybir.AluOpType.mult,
                    )

                # T1_h = attn_h^T @ Wproj_h
                for h in range(H):
                    t1p = ft1.tile([HD, C], F32, tag="t1p")
                    nc.tensor.matmul(
                        t1p[:, 0:512], lhsT=R(at_all[:, h, :]),
                        rhs=R(wprojE_sb[:, h, 0:512]), start=True, stop=True,
                    )
                    nc.tensor.matmul(
                        t1p[:, 512:C], lhsT=R(at_all[:, h, :]),
                        rhs=R(wprojE_sb[:, h, 512:C]), start=True, stop=True,
                    )
                    nc.vector.tensor_copy(out=t1_sb[:, h, :], in_=t1p)

                wpe_ctx.close()
                wvt_ctx = ExitStack()
                wvt_pool = wvt_ctx.enter_context(tc.tile_pool(name="wvt", bufs=1))
                wvT_sb = wvt_pool.tile([HD, H, C], F32R)  # Wv_h^T per head
                # transpose Wv per head: wvT_sb[:, h, cb*128:+128]
                for h in range(H):
                    for cb in range(CB):
                        wvt = fwvt.tile([HD, P], F32, tag="wvt")
                        nc.tensor.matmul(
                            wvt,
                            lhsT=R(wv_sb[:, cb, h * HD : (h + 1) * HD]),
                            rhs=R(ident),
                            start=True,
                            stop=True,
                        )
                        nc.vector.tensor_copy(
                            out=wvT_sb[:, h, cb * P : (cb + 1) * P], in_=wvt
                        )

            # Wbig[cb] = sum_h wvT_h[:, cb]^T @ T1_h
            with tc.tile_pool(name="fpsC", bufs=3, space="PSUM") as fpC:
                for cb in range(CB):
                    wbp = fpC.tile([P, C], F32, tag="wbp")
                    for h in range(H):
                        lh = R(wvT_sb[:, h, cb * P : (cb + 1) * P])
                        nc.tensor.matmul(
                            wbp[:, 0:512], lhsT=lh, rhs=R(t1_sb[:, h, 0:512]),
                            start=(h == 0), stop=(h == H - 1),
                        )
                        nc.tensor.matmul(
                            wbp[:, 512:C], lhsT=lh, rhs=R(t1_sb[:, h, 512:C]),
                            start=(h == 0), stop=(h == H - 1),
                        )
                    nc.vector.tensor_copy(out=wbig_sb[:, cb, :], in_=wbp)
            wvt_ctx.close()

        # ---------------- pass 2: y = x @ Wbig + bias ----------------
        with tc.tile_pool(name="yps", bufs=3, space="PSUM") as ypool, \
             tc.tile_pool(name="singles2", bufs=1) as singles2, \
             tc.tile_pool(name="ysb", bufs=3) as ysbpool:
            bias_sb = singles2.tile([P, C], F32)
            nc.sync.dma_start(out=bias_sb, in_=bproj[None, :].to_broadcast([P, C]))
            for i in range(nch):
                nsl_i = slice(i * P, (i + 1) * P)
                yt = ypool.tile([P, C], F32, tag="yt")
                for cb in range(CB):
                    lh = R(xt_sb[:, cb, nsl_i])
                    nc.tensor.matmul(
                        yt[:, 0:512], lhsT=lh, rhs=R(wbig_sb[:, cb, 0:512]),
                        start=(cb == 0), stop=(cb == CB - 1),
                    )
                    nc.tensor.matmul(
                        yt[:, 512:C], lhsT=lh, rhs=R(wbig_sb[:, cb, 512:C]),
                        start=(cb == 0), stop=(cb == CB - 1),
                    )
                ysb = ysbpool.tile([P, C], F32, tag="ysb")
                nc.vector.tensor_copy(out=ysb, in_=yt)
                nc.gpsimd.tensor_tensor(
                    out=ysb, in0=ysb, in1=bias_sb, op=mybir.AluOpType.add
                )
                nc.sync.dma_start(out=y[nsl_i, :], in_=ysb)

    nc.compile()
    return nc


def kernel(x, Wqkv, temperature, Wproj, bproj):
    from concourse.bass_utils import run_bass_kernel_spmd

    B = x.shape[0]
    key = "nc"
    if key not in _CACHE:
        _CACHE[key] = _build()
    nc = _CACHE[key]

    wqkv = np.ascontiguousarray(np.asarray(Wqkv, dtype=np.float32))
    temp = np.ascontiguousarray(np.asarray(temperature, dtype=np.float32).reshape(H))
    wproj = np.ascontiguousarray(np.asarray(Wproj, dtype=np.float32))
    bias = np.ascontiguousarray(np.asarray(bproj, dtype=np.float32))
    in_maps = [
        {
            "x": np.ascontiguousarray(np.asarray(x[b], dtype=np.float32)),
            "wqkv": wqkv,
            "temp": temp,
            "wproj": wproj,
            "bproj": bias,
        }
        for b in range(B)
    ]
    res = run_bass_kernel_spmd(nc, in_maps, core_ids=list(range(B)))
    out = np.stack([res.results[b]["y"] for b in range(B)], axis=0)
    return out.astype(np.float32)


if __name__ == "__main__":
    rng = np.random.default_rng(0)
    inputs = {
        "x": rng.standard_normal((8, N, C), dtype=np.float32),
        "Wqkv": (rng.standard_normal((C, NC3)) / np.sqrt(C)).astype(np.float32),
        "temperature": np.ones((H, 1, 1), dtype=np.float32),
        "Wproj": (rng.standard_normal((C, C)) / np.sqrt(C)).astype(np.float32),
        "bproj": (rng.standard_normal(C) * 0.01).astype(np.float32),
    }
    out = kernel(**inputs)
    print(out.shape, out.dtype)



# revision 10
# speedup vs baseline: 1.6621x; 1.6621x over previous
"""ChannelAttention Trainium2 Bass kernel.

Data-parallel over batch: 8 batches -> 8 NeuronCores, zero communication.

Key algebra: q,k are never materialized.  With G = x^T x  [C, C]:
  gram_h   = q_h^T k_h = Wq_h^T G Wk_h
  ||q_d||^2 = diag(Wq_h^T G Wq_h)  (same for k)
so pass 1 only accumulates G (upper triangle, symmetric) from streamed
token chunks.  The finalize runs on [768 x 1536] matrices:
  A2 = G @ [Wq | Wk]; gram_h = Wq_h^T A2k_h; sumsq = colsum(W2 * A2)
then softmax and the fused output matrix
  Wbig = sum_h Wv_h @ attn_h^T @ Wproj_h          [C, C]
Pass 2 computes y^T = Wbig^T @ x^T + b in bf16, streaming a
host-supplied bf16 x^T (no on-device transposes); host transposes back.

Weight/output DMAs ride the scalar HWDGE ring so the x streams on the
sync ring are never queued behind them.
"""

import sys

if "/opt/trn_rl_repo" not in sys.path:
    sys.path.insert(0, "/opt/trn_rl_repo")

import numpy as np

N, C, H, HD = 4096, 768, 8, 96
C2 = 2 * C
NC3 = 3 * C
EPS = 1e-12
P = 128
CB = C // P            # 6 channel blocks
NCH = N // P           # 32 token chunks

# upper-triangle block packing: block (r, c), r <= c, index b -> bank b//4,
# column offset (b%4)*128 inside PSUM tiles of [128, 512]
_STARTS = [0, 6, 11, 15, 18, 20]
# per row: list of (bank, offset, c0, ncols) matmul runs covering cols c0..
_G_RUNS = {
    0: [(0, 0, 0, 512), (1, 0, 4, 256)],
    1: [(1, 256, 1, 256), (2, 0, 3, 384)],
    2: [(2, 384, 2, 128), (3, 0, 3, 384)],
    3: [(3, 384, 3, 128), (4, 0, 4, 256)],
    4: [(4, 256, 4, 256)],
    5: [(5, 0, 5, 128)],
}

_CACHE = {}


def _blk(b):
    return b // 4, (b % 4) * P


def _build(dbg=False):
    import concourse.bacc as bacc
    import concourse.tile as tile
    import concourse.mybir as mybir
    from concourse.masks import make_identity
    from contextlib import ExitStack

    F32 = mybir.dt.float32
    F32R = mybir.dt.float32r
    BF16 = mybir.dt.bfloat16

    def R(ap):
        return ap.bitcast(F32R)

    nc = bacc.Bacc("TRN2", target_bir_lowering=False, debug=False, num_devices=8)
    x = nc.dram_tensor("x", [N, C], F32, kind="ExternalInput")
    xt = nc.dram_tensor("xt", [C, N], BF16, kind="ExternalInput")
    w2 = nc.dram_tensor("w2", [C, C2], F32, kind="ExternalInput")
    wvt = nc.dram_tensor("wvt", [HD, H, C], F32, kind="ExternalInput")
    wpe = nc.dram_tensor("wpe", [HD, H, C], F32, kind="ExternalInput")
    temp = nc.dram_tensor("temp", [H], F32, kind="ExternalInput")
    biasE = nc.dram_tensor("biasE", [P, CB], F32, kind="ExternalInput")
    yt = nc.dram_tensor("yt", [C, N], BF16, kind="ExternalOutput")
    if dbg:
        dbg_g = nc.dram_tensor("dbg_g", [P, CB, C], F32, kind="ExternalOutput")
        dbg_ss = nc.dram_tensor("dbg_ss", [HD, 2 * H], F32, kind="ExternalOutput")
        dbg_s = nc.dram_tensor("dbg_s", [HD, 2 * H], F32, kind="ExternalOutput")
        dbg_at = nc.dram_tensor("dbg_at", [HD, H, HD], F32, kind="ExternalOutput")
        dbg_wb = nc.dram_tensor("dbg_wb", [P, CB, C], BF16, kind="ExternalOutput")
        dbg_t1 = nc.dram_tensor("dbg_t1", [HD, H, C], F32, kind="ExternalOutput")
        dbg_pp = nc.dram_tensor("dbg_pp", [P, C2], F32, kind="ExternalOutput")

    with tile.TileContext(nc) as tc, ExitStack() as ctx:
        singles = ctx.enter_context(tc.tile_pool(name="singles", bufs=1))
        ident_f = singles.tile([P, P], F32)
        ident_r = singles.tile([P, P], F32R)
        ones_f = singles.tile([P, HD], F32)
        ones1 = singles.tile([P, 1], F32R)
        temp_sb = singles.tile([HD, H], F32)
        bias_sb = singles.tile([P, CB], F32)
        s_sb = singles.tile([HD, 2 * H], F32)
        sumsq_sb = singles.tile([HD, 2 * H], F32)
        atsb = singles.tile([HD, H, HD], F32R)
        wbig_sb = singles.tile([P, CB, C], BF16)

        make_identity(nc, ident_f)
        nc.vector.tensor_copy(out=ident_r, in_=ident_f)
        nc.vector.memset(ones_f, 1.0)
        nc.vector.tensor_copy(out=ones1, in_=ones_f[:, 0:1])
        nc.scalar.dma_start(out=temp_sb, in_=temp[None, :].to_broadcast([HD, H]))
        nc.scalar.dma_start(out=bias_sb, in_=biasE[:, :])

        # finalize weights stream on the scalar ring during pass 1
        wvt_sb = singles.tile([HD, H, C], F32R)
        wpe_sb = singles.tile([HD, H, C], F32R)
        c_ctx = ExitStack()
        cpool = c_ctx.enter_context(tc.tile_pool(name="xtcp", bufs=3))
        ypool = c_ctx.enter_context(tc.tile_pool(name="ysbp", bufs=4))

        w2_ctx = ExitStack()
        w2_pool = w2_ctx.enter_context(tc.tile_pool(name="w2p", bufs=1))
        w2_sb = w2_pool.tile([P, CB, C2], F32R)
        nc.scalar.dma_start(
            out=w2_sb, in_=R(w2.rearrange("(cb p) j -> p cb j", p=P))
        )
        nc.scalar.dma_start(out=wvt_sb, in_=R(wvt[:, :, :]))
        nc.scalar.dma_start(out=wpe_sb, in_=R(wpe[:, :, :]))

        # ---------------- pass 1: G = x^T x (upper triangle) ----------------
        gram_ctx = ExitStack()
        gram_pool = gram_ctx.enter_context(
            tc.tile_pool(name="gps", bufs=1, space="PSUM")
        )
        gtile = [
            gram_pool.tile([P, 512], F32, tag=f"g{i}", name=f"g{i}")
            for i in range(6)
        ]

        with tc.tile_pool(name="p1", bufs=6) as p1pool:
            for i in range(NCH):
                xc = p1pool.tile([P, C], F32R, tag="xc")
                nc.sync.dma_start(out=xc, in_=R(x[i * P : (i + 1) * P, :]))
                for r in range(CB):
                    lh = xc[:, r * P : (r + 1) * P]
                    for (bank, off, c0, ncols) in _G_RUNS[r]:
                        nc.tensor.matmul(
                            gtile[bank][:, off : off + ncols],
                            lhsT=lh,
                            rhs=xc[:, c0 * P : c0 * P + ncols],
                            start=(i == 0 and off == 0),
                            stop=(i == NCH - 1),
                            skip_group_check=True,
                        )

        # ---------------- finalize ----------------
        fs_ctx = ExitStack()
        fsb = fs_ctx.enter_context(tc.tile_pool(name="fsb", bufs=2))
        fsb2 = fs_ctx.enter_context(tc.tile_pool(name="fsb2", bufs=1))
        a2pool_sb = fs_ctx.enter_context(tc.tile_pool(name="a2sb", bufs=2))
        pp = fs_ctx.enter_context(tc.tile_pool(name="ppp", bufs=1)).tile(
            [P, C2], F32R
        )

        gsb_ctx = ExitStack()
        gsb_pool = gsb_ctx.enter_context(tc.tile_pool(name="gsbp", bufs=1))
        gsb = gsb_pool.tile([P, CB, C], F32R)

        # PSUM -> SBUF upper blocks (split DVE/ACT), mirror via PE matmul
        for r in range(CB):
            for c in range(r, CB):
                bank, off = _blk(_STARTS[r] + c - r)
                if (r + c) % 2 == 0:
                    nc.vector.tensor_copy(
                        out=gsb[:, r, c * P : (c + 1) * P],
                        in_=gtile[bank][:, off : off + P],
                    )
                else:
                    nc.scalar.copy(
                        out=gsb[:, r, c * P : (c + 1) * P],
                        in_=gtile[bank][:, off : off + P],
                    )
        gram_ctx.close()

        with tc.tile_pool(name="tpps", bufs=2, space="PSUM") as tppool:
            for r in range(CB):
                for c in range(r + 1, CB):
                    tp = tppool.tile([P, P], F32, tag="tp")
                    nc.tensor.matmul(
                        tp,
                        lhsT=gsb[:, r, c * P : (c + 1) * P],
                        rhs=ident_r,
                        start=True,
                        stop=True,
                    )
                    nc.vector.tensor_copy(
                        out=gsb[:, c, r * P : (r + 1) * P], in_=tp
                    )

        if dbg:
            nc.scalar.dma_start(out=dbg_g[:, :, :], in_=gsb.bitcast(F32))

        # A2 = G @ [Wq | Wk] row-by-row, with per-row gram matmuls and
        # sum-of-squares products interleaved so PE stays dense.
        at_ctx = ExitStack()
        atpool = at_ctx.enter_context(
            tc.tile_pool(name="atps", bufs=1, space="PSUM")
        )
        at1 = atpool.tile([HD, 5 * HD], F32, tag="at1", name="at1")
        at2 = atpool.tile([HD, 3 * HD], F32, tag="at2", name="at2")

        with tc.tile_pool(name="a2ps", bufs=2, space="PSUM") as a2pool:
            for r in range(CB):
                a2p = a2pool.tile([P, C2], F32, tag="a2p")
                for cb in range(CB):
                    lh = gsb[:, cb, r * P : (r + 1) * P]
                    for js in range(3):
                        nc.tensor.matmul(
                            a2p[:, js * 512 : (js + 1) * 512],
                            lhsT=lh,
                            rhs=w2_sb[:, cb, js * 512 : (js + 1) * 512],
                            start=(cb == 0),
                            stop=(cb == CB - 1),
                            skip_group_check=True,
                        )
                a2row = a2pool_sb.tile([P, C2], F32R, tag="a2row")
                nc.vector.tensor_copy(out=a2row, in_=a2p)
                # gram contributions of this row
                for h in range(H):
                    bank = at1 if h < 5 else at2
                    co = HD * h if h < 5 else HD * (h - 5)
                    nc.tensor.matmul(
                        bank[:, co : co + HD],
                        lhsT=w2_sb[:, r, h * HD : (h + 1) * HD],
                        rhs=a2row[:, C + h * HD : C + (h + 1) * HD],
                        start=(r == 0 and h in (0, 5)),
                        stop=(r == CB - 1),
                        skip_group_check=True,
                    )
                # sum-of-squares partial products
                if r == 0:
                    nc.vector.tensor_tensor(
                        out=pp, in0=w2_sb[:, 0, :], in1=a2row,
                        op=mybir.AluOpType.mult,
                    )
                else:
                    pt = fsb.tile([P, C2], F32R, tag="pt")
                    nc.vector.tensor_tensor(
                        out=pt, in0=w2_sb[:, r, :], in1=a2row,
                        op=mybir.AluOpType.mult,
                    )
                    nc.vector.tensor_tensor(
                        out=pp, in0=pp, in1=pt, op=mybir.AluOpType.add
                    )
        gsb_ctx.close()

        with tc.tile_pool(name="ssps", bufs=1, space="PSUM") as sspool:
            ssp = [
                sspool.tile([1, 512], F32, tag=f"ss{j}", name=f"ss{j}")
                for j in range(3)
            ]
            sqp = sspool.tile([HD, 2 * H], F32, tag="sqp", name="sqp")
            for js in range(3):
                nc.tensor.matmul(
                    ssp[js],
                    lhsT=ones1,
                    rhs=pp[:, js * 512 : (js + 1) * 512],
                    start=True,
                    stop=True,
                )
            ssrow = fsb2.tile([1, C2], F32, tag="ssrow")
            for js in range(3):
                nc.vector.tensor_copy(
                    out=ssrow[:, js * 512 : (js + 1) * 512], in_=ssp[js]
                )
            # row -> columns [96, 16] via K=1 fp32 matmuls
            for t in range(2):
                for h in range(H):
                    j = t * H + h
                    f0 = t * C + h * HD
                    nc.tensor.matmul(
                        sqp[:, j : j + 1],
                        lhsT=ssrow[0:1, f0 : f0 + HD],
                        rhs=ones_f[0:1, 0:1],
                        start=(j == 0),
                        stop=(j == 2 * H - 1),
                        skip_group_check=True,
                    )
            nc.vector.tensor_copy(out=sumsq_sb, in_=sqp)

            # s = 1/max(sqrt(ss), eps); temperature folded into s_q
            nc.scalar.sqrt(out=s_sb, in_=sumsq_sb)
            nc.vector.tensor_scalar_max(s_sb, s_sb, EPS)
            nc.vector.reciprocal(out=s_sb, in_=s_sb)
            nc.vector.tensor_tensor(
                out=s_sb[:, 0:H],
                in0=s_sb[:, 0:H],
                in1=temp_sb,
                op=mybir.AluOpType.mult,
            )

        # combined scale [d,h,e] = s_q[d,h] * s_k[h,e] via ones96^T @ diag
        if True:
            with tc.tile_pool(name="skps", bufs=1, space="PSUM") as skpool:
                ones96 = fsb2.tile([HD, HD], F32R, tag="ones96")
                nc.vector.tensor_copy(out=ones96, in_=ones_f[0:HD, :])
                diag_all = fsb2.tile([HD, H, HD], F32R, tag="diag_all")
                nc.vector.tensor_tensor(
                    out=diag_all,
                    in0=ident_r[0:HD, None, 0:HD].to_broadcast([HD, H, HD]),
                    in1=s_sb[:, H : 2 * H, None].to_broadcast([HD, H, HD]),
                    op=mybir.AluOpType.mult,
                )
                skp = skpool.tile([HD, H * HD], F32, tag="skp")
                df = R(diag_all).rearrange("p h e -> p (h e)")
                nc.tensor.matmul(
                    skp[:, 0:512], lhsT=ones96, rhs=df[:, 0:512],
                    start=True, stop=True,
                )
                nc.tensor.matmul(
                    skp[:, 512:768], lhsT=ones96, rhs=df[:, 512:768],
                    start=True, stop=True,
                )
                skrep = fsb2.tile([HD, H, HD], F32, tag="skrep")
                nc.vector.tensor_copy(
                    out=skrep.rearrange("p h e -> p (h e)"), in_=skp
                )
                nc.vector.tensor_tensor(
                    out=skrep,
                    in0=skrep,
                    in1=s_sb[:, 0:H, None].to_broadcast([HD, H, HD]),
                    op=mybir.AluOpType.mult,
                )

            # softmax per head-group (no max subtraction: |attn| <= temp),
            # T1_h = attn_h^T @ Wproj_h follows each group on PE
            t1_ctx = ExitStack()
            t1_pool = t1_ctx.enter_context(tc.tile_pool(name="t1p", bufs=1))
            t1_sb = t1_pool.tile([HD, H, C], F32R)
            with tc.tile_pool(name="t1ps", bufs=2, space="PSUM") as t1ps:
                for g, (h0, nh) in enumerate(((0, 5), (5, 3))):
                    bank = at1 if g == 0 else at2
                    ga = atsb[:, h0 : h0 + nh, :]
                    nc.vector.tensor_copy(
                        out=ga.rearrange("p h e -> p (h e)"), in_=bank
                    )
                    nc.vector.tensor_tensor(
                        out=ga, in0=ga, in1=skrep[:, h0 : h0 + nh, :],
                        op=mybir.AluOpType.mult,
                    )
                    nc.scalar.activation(
                        out=ga, in_=ga,
                        func=mybir.ActivationFunctionType.Exp,
                        bias=0.0, scale=1.0,
                    )
                    rsum = fsb.tile([HD, H], F32, tag="rsum")
                    nc.vector.tensor_reduce(
                        out=rsum[:, 0:nh], in_=ga, axis=mybir.AxisListType.X,
                        op=mybir.AluOpType.add,
                    )
                    nc.vector.reciprocal(out=rsum[:, 0:nh], in_=rsum[:, 0:nh])
                    nc.vector.tensor_tensor(
                        out=ga, in0=ga,
                        in1=rsum[:, 0:nh, None].to_broadcast([HD, nh, HD]),
                        op=mybir.AluOpType.mult,
                    )
                    for h in range(h0, h0 + nh):
                        t1p = t1ps.tile([HD, C], F32, tag="t1p")
                        lh = atsb[:, h, :]
                        nc.tensor.matmul(
                            t1p[:, 0:512], lhsT=lh, rhs=wpe_sb[:, h, 0:512],
                            start=True, stop=True,
                        )
                        nc.tensor.matmul(
                            t1p[:, 512:C], lhsT=lh, rhs=wpe_sb[:, h, 512:C],
                            start=True, stop=True,
                        )
                        nc.vector.tensor_copy(out=t1_sb[:, h, :], in_=t1p)
        if dbg:
            nc.scalar.dma_start(out=dbg_at[:, :, :], in_=atsb.bitcast(F32))
            nc.scalar.dma_start(out=dbg_t1[:, :, :], in_=t1_sb.bitcast(F32))
        at_ctx.close()

        # Wbig = sum_h Wv_h @ T1_h
        with tc.tile_pool(name="wbps", bufs=2, space="PSUM") as wbps:
            for m in range(CB):
                wbp = wbps.tile([P, C], F32, tag="wbp")
                for h in range(H):
                    lh = wvt_sb[:, h, m * P : (m + 1) * P]
                    nc.tensor.matmul(
                        wbp[:, 0:512], lhsT=lh, rhs=t1_sb[:, h, 0:512],
                        start=(h == 0), stop=(h == H - 1),
                        skip_group_check=True,
                    )
                    nc.tensor.matmul(
                        wbp[:, 512:C], lhsT=lh, rhs=t1_sb[:, h, 512:C],
                        start=(h == 0), stop=(h == H - 1),
                        skip_group_check=True,
                    )
                nc.vector.tensor_copy(out=wbig_sb[:, m, :], in_=wbp)
        if dbg:
            nc.scalar.dma_start(out=dbg_wb[:, :, :], in_=wbig_sb)
        t1_ctx.close()
        fs_ctx.close()
        w2_ctx.close()

        # ---------------- pass 2: y^T = Wbig^T @ x^T + b (bf16) -------------
        with tc.tile_pool(name="yps", bufs=8, space="PSUM") as yps:
            for n in range(8):
                nsl = slice(n * 512, (n + 1) * 512)
                xtc = cpool.tile([P, CB, 512], BF16, tag="xtc")
                nc.sync.dma_start(
                    out=xtc,
                    in_=xt[:, nsl].rearrange("(cb p) m -> p cb m", p=P),
                )
                for co in range(CB):
                    ypb = yps.tile([P, 512], F32, tag="ypb")
                    for ci in range(CB):
                        nc.tensor.matmul(
                            ypb,
                            lhsT=wbig_sb[:, ci, co * P : (co + 1) * P],
                            rhs=xtc[:, ci, :],
                            start=(ci == 0),
                            stop=(ci == CB - 1),
                        )
                    ysb = ypool.tile([P, 512], BF16, tag="ysb")
                    nc.vector.tensor_tensor(
                        out=ysb,
                        in0=ypb,
                        in1=bias_sb[:, co : co + 1].to_broadcast([P, 512]),
                        op=mybir.AluOpType.add,
                    )
                    nc.scalar.dma_start(
                        out=yt[co * P : (co + 1) * P, nsl], in_=ysb
                    )
        c_ctx.close()

    nc.compile()
    return nc


def prep_inputs(x, Wqkv, temperature, Wproj, bproj):
    import ml_dtypes

    B = x.shape[0]
    wqkv = np.asarray(Wqkv, dtype=np.float32)
    w2 = np.ascontiguousarray(wqkv[:, :C2])
    wvt = np.ascontiguousarray(
        wqkv[:, C2:].T.reshape(H, HD, C).transpose(1, 0, 2)
    )
    wpe = np.ascontiguousarray(
        np.asarray(Wproj, dtype=np.float32).reshape(H, HD, C).transpose(1, 0, 2)
    )
    temp = np.ascontiguousarray(np.asarray(temperature, dtype=np.float32).reshape(H))
    biasE = np.ascontiguousarray(
        np.asarray(bproj, dtype=np.float32).reshape(CB, P).T
    )
    xf = np.asarray(x, dtype=np.float32)
    in_maps = [
        {
            "x": np.ascontiguousarray(xf[b]),
            "xt": np.ascontiguousarray(xf[b].T.astype(ml_dtypes.bfloat16)),
            "w2": w2,
            "wvt": wvt,
            "wpe": wpe,
            "temp": temp,
            "biasE": biasE,
        }
        for b in range(B)
    ]
    return in_maps


def kernel(x, Wqkv, temperature, Wproj, bproj):
    from concourse.bass_utils import run_bass_kernel_spmd

    B = x.shape[0]
    key = "nc"
    if key not in _CACHE:
        _CACHE[key] = _build()
    nc = _CACHE[key]

    in_maps = prep_inputs(x, Wqkv, temperature, Wproj, bproj)
    res = run_bass_kernel_spmd(nc, in_maps, core_ids=list(range(B)))
    out = np.stack(
        [res.results[b]["yt"].T.astype(np.float32) for b in range(B)], axis=0
    )
    return np.ascontiguousarray(out)


if __name__ == "__main__":
    rng = np.random.default_rng(0)
    inputs = {
        "x": rng.standard_normal((8, N, C), dtype=np.float32),
        "Wqkv": (rng.standard_normal((C, NC3)) / np.sqrt(C)).astype(np.float32),
        "temperature": np.ones((H, 1, 1), dtype=np.float32),
        "Wproj": (rng.standard_normal((C, C)) / np.sqrt(C)).astype(np.float32),
        "bproj": (rng.standard_normal(C) * 0.01).astype(np.float32),
    }
    out = kernel(**inputs)
    print(out.shape, out.dtype)


# revision 11
# speedup vs baseline: 1.7136x; 1.0309x over previous
"""ChannelAttention Trainium2 Bass kernel.

Data-parallel over batch: 8 batches -> 8 NeuronCores, zero communication.

Key algebra: q,k are never materialized.  With G = x^T x  [C, C]:
  gram_h   = q_h^T k_h = Wq_h^T G Wk_h
  ||q_d||^2 = diag(Wq_h^T G Wq_h)  (same for k)
so pass 1 only accumulates G (upper triangle, symmetric) from streamed
token chunks.  The finalize runs on [768 x 1536] matrices:
  A2 = G @ [Wq | Wk]; gram_h = Wq_h^T A2k_h; sumsq = colsum(W2 * A2)
then softmax and the fused output matrix
  Wbig = sum_h Wv_h @ attn_h^T @ Wproj_h          [C, C]
Pass 2 computes y^T = Wbig^T @ x^T + b in bf16, streaming a
host-supplied bf16 x^T (no on-device transposes); host transposes back.

Weight/output DMAs ride the scalar HWDGE ring so the x streams on the
sync ring are never queued behind them.
"""

import sys

if "/opt/trn_rl_repo" not in sys.path:
    sys.path.insert(0, "/opt/trn_rl_repo")

import numpy as np

N, C, H, HD = 4096, 768, 8, 96
C2 = 2 * C
NC3 = 3 * C
EPS = 1e-12
P = 128
CB = C // P            # 6 channel blocks
NCH = N // P           # 32 token chunks

# upper-triangle block packing: block (r, c), r <= c, index b -> bank b//4,
# column offset (b%4)*128 inside PSUM tiles of [128, 512]
_STARTS = [0, 6, 11, 15, 18, 20]
# per row: list of (bank, offset, c0, ncols) matmul runs covering cols c0..
_G_RUNS = {
    0: [(0, 0, 0, 512), (1, 0, 4, 256)],
    1: [(1, 256, 1, 256), (2, 0, 3, 384)],
    2: [(2, 384, 2, 128), (3, 0, 3, 384)],
    3: [(3, 384, 3, 128), (4, 0, 4, 256)],
    4: [(4, 256, 4, 256)],
    5: [(5, 0, 5, 128)],
}

_CACHE = {}


def _blk(b):
    return b // 4, (b % 4) * P


def _build(dbg=False):
    import concourse.bacc as bacc
    import concourse.tile as tile
    import concourse.mybir as mybir
    from concourse.masks import make_identity
    from contextlib import ExitStack

    F32 = mybir.dt.float32
    F32R = mybir.dt.float32r
    BF16 = mybir.dt.bfloat16

    def R(ap):
        return ap.bitcast(F32R)

    nc = bacc.Bacc("TRN2", target_bir_lowering=False, debug=False, num_devices=8)
    x = nc.dram_tensor("x", [N, C], BF16, kind="ExternalInput")
    xt = nc.dram_tensor("xt", [C, N], BF16, kind="ExternalInput")
    w2 = nc.dram_tensor("w2", [C, C2], F32, kind="ExternalInput")
    wvt = nc.dram_tensor("wvt", [HD, H, C], F32, kind="ExternalInput")
    wpe = nc.dram_tensor("wpe", [HD, H, C], F32, kind="ExternalInput")
    temp = nc.dram_tensor("temp", [H], F32, kind="ExternalInput")
    biasE = nc.dram_tensor("biasE", [P, CB], F32, kind="ExternalInput")
    yt = nc.dram_tensor("yt", [C, N], BF16, kind="ExternalOutput")
    if dbg:
        dbg_g = nc.dram_tensor("dbg_g", [P, CB, C], F32, kind="ExternalOutput")
        dbg_ss = nc.dram_tensor("dbg_ss", [HD, 2 * H], F32, kind="ExternalOutput")
        dbg_s = nc.dram_tensor("dbg_s", [HD, 2 * H], F32, kind="ExternalOutput")
        dbg_at = nc.dram_tensor("dbg_at", [HD, H, HD], F32, kind="ExternalOutput")
        dbg_wb = nc.dram_tensor("dbg_wb", [P, CB, C], BF16, kind="ExternalOutput")
        dbg_t1 = nc.dram_tensor("dbg_t1", [HD, H, C], F32, kind="ExternalOutput")
        dbg_pp = nc.dram_tensor("dbg_pp", [P, C2], F32, kind="ExternalOutput")

    with tile.TileContext(nc) as tc, ExitStack() as ctx:
        singles = ctx.enter_context(tc.tile_pool(name="singles", bufs=1))
        ident_f = singles.tile([P, P], F32)
        ident_r = singles.tile([P, P], F32R)
        ones_f = singles.tile([P, HD], F32)
        ones1 = singles.tile([P, 1], F32R)
        temp_sb = singles.tile([HD, H], F32)
        bias_sb = singles.tile([P, CB], F32)
        s_sb = singles.tile([HD, 2 * H], F32)
        sumsq_sb = singles.tile([HD, 2 * H], F32)
        atsb = singles.tile([HD, H, HD], F32R)
        wbig_sb = singles.tile([P, CB, C], BF16)

        make_identity(nc, ident_f)
        nc.vector.tensor_copy(out=ident_r, in_=ident_f)
        nc.vector.memset(ones_f, 1.0)
        nc.vector.tensor_copy(out=ones1, in_=ones_f[:, 0:1])
        nc.scalar.dma_start(out=temp_sb, in_=temp[None, :].to_broadcast([HD, H]))
        nc.scalar.dma_start(out=bias_sb, in_=biasE[:, :])

        # finalize weights stream on the scalar ring during pass 1
        wvt_sb = singles.tile([HD, H, C], F32R)
        wpe_sb = singles.tile([HD, H, C], F32R)
        c_ctx = ExitStack()
        cpool = c_ctx.enter_context(tc.tile_pool(name="xtcp", bufs=3))
        ypool = c_ctx.enter_context(tc.tile_pool(name="ysbp", bufs=4))

        w2_ctx = ExitStack()
        w2_pool = w2_ctx.enter_context(tc.tile_pool(name="w2p", bufs=1))
        w2_sb = w2_pool.tile([P, CB, C2], F32R)
        nc.scalar.dma_start(
            out=w2_sb, in_=R(w2.rearrange("(cb p) j -> p cb j", p=P))
        )
        nc.scalar.dma_start(out=wvt_sb, in_=R(wvt[:, :, :]))
        nc.scalar.dma_start(out=wpe_sb, in_=R(wpe[:, :, :]))

        # ---------------- pass 1: G = x^T x (upper triangle) ----------------
        gram_ctx = ExitStack()
        gram_pool = gram_ctx.enter_context(
            tc.tile_pool(name="gps", bufs=1, space="PSUM")
        )
        gtile = [
            gram_pool.tile([P, 512], F32, tag=f"g{i}", name=f"g{i}")
            for i in range(6)
        ]

        with tc.tile_pool(name="p1", bufs=6) as p1pool:
            for i in range(NCH):
                xc = p1pool.tile([P, C], BF16, tag="xc")
                nc.sync.dma_start(out=xc, in_=x[i * P : (i + 1) * P, :])
                for r in range(CB):
                    lh = xc[:, r * P : (r + 1) * P]
                    for (bank, off, c0, ncols) in _G_RUNS[r]:
                        nc.tensor.matmul(
                            gtile[bank][:, off : off + ncols],
                            lhsT=lh,
                            rhs=xc[:, c0 * P : c0 * P + ncols],
                            start=(i == 0 and off == 0),
                            stop=(i == NCH - 1),
                            skip_group_check=True,
                        )

        # ---------------- finalize ----------------
        fs_ctx = ExitStack()
        fsb = fs_ctx.enter_context(tc.tile_pool(name="fsb", bufs=2))
        fsb2 = fs_ctx.enter_context(tc.tile_pool(name="fsb2", bufs=1))
        a2pool_sb = fs_ctx.enter_context(tc.tile_pool(name="a2sb", bufs=2))
        pp = fs_ctx.enter_context(tc.tile_pool(name="ppp", bufs=1)).tile(
            [P, C2], F32R
        )

        gsb_ctx = ExitStack()
        gsb_pool = gsb_ctx.enter_context(tc.tile_pool(name="gsbp", bufs=1))
        gsb = gsb_pool.tile([P, CB, C], F32R)

        # PSUM -> SBUF upper blocks (split DVE/ACT), mirror via PE matmul
        for r in range(CB):
            for c in range(r, CB):
                bank, off = _blk(_STARTS[r] + c - r)
                if (r + c) % 2 == 0:
                    nc.vector.tensor_copy(
                        out=gsb[:, r, c * P : (c + 1) * P],
                        in_=gtile[bank][:, off : off + P],
                    )
                else:
                    nc.scalar.copy(
                        out=gsb[:, r, c * P : (c + 1) * P],
                        in_=gtile[bank][:, off : off + P],
                    )
        gram_ctx.close()

        with tc.tile_pool(name="tpps", bufs=2, space="PSUM") as tppool:
            for r in range(CB):
                for c in range(r + 1, CB):
                    tp = tppool.tile([P, P], F32, tag="tp")
                    nc.tensor.matmul(
                        tp,
                        lhsT=gsb[:, r, c * P : (c + 1) * P],
                        rhs=ident_r,
                        start=True,
                        stop=True,
                    )
                    nc.vector.tensor_copy(
                        out=gsb[:, c, r * P : (r + 1) * P], in_=tp
                    )

        if dbg:
            nc.scalar.dma_start(out=dbg_g[:, :, :], in_=gsb.bitcast(F32))

        # A2 = G @ [Wq | Wk] row-by-row, with per-row gram matmuls and
        # sum-of-squares products interleaved so PE stays dense.
        at_ctx = ExitStack()
        atpool = at_ctx.enter_context(
            tc.tile_pool(name="atps", bufs=1, space="PSUM")
        )
        at1 = atpool.tile([HD, 5 * HD], F32, tag="at1", name="at1")
        at2 = atpool.tile([HD, 3 * HD], F32, tag="at2", name="at2")

        with tc.tile_pool(name="a2ps", bufs=2, space="PSUM") as a2pool:
            for r in range(CB):
                a2p = a2pool.tile([P, C2], F32, tag="a2p")
                for cb in range(CB):
                    lh = gsb[:, cb, r * P : (r + 1) * P]
                    for js in range(3):
                        nc.tensor.matmul(
                            a2p[:, js * 512 : (js + 1) * 512],
                            lhsT=lh,
                            rhs=w2_sb[:, cb, js * 512 : (js + 1) * 512],
                            start=(cb == 0),
                            stop=(cb == CB - 1),
                            skip_group_check=True,
                        )
                a2row = a2pool_sb.tile([P, C2], F32R, tag="a2row")
                nc.vector.tensor_copy(out=a2row, in_=a2p)
                # gram contributions of this row
                for h in range(H):
                    bank = at1 if h < 5 else at2
                    co = HD * h if h < 5 else HD * (h - 5)
                    nc.tensor.matmul(
                        bank[:, co : co + HD],
                        lhsT=w2_sb[:, r, h * HD : (h + 1) * HD],
                        rhs=a2row[:, C + h * HD : C + (h + 1) * HD],
                        start=(r == 0 and h in (0, 5)),
                        stop=(r == CB - 1),
                        skip_group_check=True,
                    )
                # sum-of-squares partial products
                if r == 0:
                    nc.vector.tensor_tensor(
                        out=pp, in0=w2_sb[:, 0, :], in1=a2row,
                        op=mybir.AluOpType.mult,
                    )
                else:
                    pt = fsb.tile([P, C2], F32R, tag="pt")
                    nc.vector.tensor_tensor(
                        out=pt, in0=w2_sb[:, r, :], in1=a2row,
                        op=mybir.AluOpType.mult,
                    )
                    nc.vector.tensor_tensor(
                        out=pp, in0=pp, in1=pt, op=mybir.AluOpType.add
                    )
        gsb_ctx.close()

        with tc.tile_pool(name="ssps", bufs=1, space="PSUM") as sspool:
            ssp = [
                sspool.tile([1, 512], F32, tag=f"ss{j}", name=f"ss{j}")
                for j in range(3)
            ]
            sqp = sspool.tile([HD, 2 * H], F32, tag="sqp", name="sqp")
            for js in range(3):
                nc.tensor.matmul(
                    ssp[js],
                    lhsT=ones1,
                    rhs=pp[:, js * 512 : (js + 1) * 512],
                    start=True,
                    stop=True,
                )
            ssrow = fsb2.tile([1, C2], F32, tag="ssrow")
            for js in range(3):
                nc.vector.tensor_copy(
                    out=ssrow[:, js * 512 : (js + 1) * 512], in_=ssp[js]
                )
            # row -> columns [96, 16] via K=1 fp32 matmuls
            for t in range(2):
                for h in range(H):
                    j = t * H + h
                    f0 = t * C + h * HD
                    nc.tensor.matmul(
                        sqp[:, j : j + 1],
                        lhsT=ssrow[0:1, f0 : f0 + HD],
                        rhs=ones_f[0:1, 0:1],
                        start=(j == 0),
                        stop=(j == 2 * H - 1),
                        skip_group_check=True,
                    )
            nc.vector.tensor_copy(out=sumsq_sb, in_=sqp)

            # s = 1/max(sqrt(ss), eps); temperature folded into s_q
            nc.scalar.sqrt(out=s_sb, in_=sumsq_sb)
            nc.vector.tensor_scalar_max(s_sb, s_sb, EPS)
            nc.vector.reciprocal(out=s_sb, in_=s_sb)
            nc.vector.tensor_tensor(
                out=s_sb[:, 0:H],
                in0=s_sb[:, 0:H],
                in1=temp_sb,
                op=mybir.AluOpType.mult,
            )

        # combined scale [d,h,e] = s_q[d,h] * s_k[h,e] via ones96^T @ diag
        if True:
            with tc.tile_pool(name="skps", bufs=1, space="PSUM") as skpool:
                ones96 = fsb2.tile([HD, HD], F32R, tag="ones96")
                nc.vector.tensor_copy(out=ones96, in_=ones_f[0:HD, :])
                diag_all = fsb2.tile([HD, H, HD], F32R, tag="diag_all")
                nc.vector.tensor_tensor(
                    out=diag_all,
                    in0=ident_r[0:HD, None, 0:HD].to_broadcast([HD, H, HD]),
                    in1=s_sb[:, H : 2 * H, None].to_broadcast([HD, H, HD]),
                    op=mybir.AluOpType.mult,
                )
                skp = skpool.tile([HD, H * HD], F32, tag="skp")
                df = R(diag_all).rearrange("p h e -> p (h e)")
                nc.tensor.matmul(
                    skp[:, 0:512], lhsT=ones96, rhs=df[:, 0:512],
                    start=True, stop=True,
                )
                nc.tensor.matmul(
                    skp[:, 512:768], lhsT=ones96, rhs=df[:, 512:768],
                    start=True, stop=True,
                )
                skrep = fsb2.tile([HD, H, HD], F32, tag="skrep")
                nc.vector.tensor_copy(
                    out=skrep.rearrange("p h e -> p (h e)"), in_=skp
                )
                nc.vector.tensor_tensor(
                    out=skrep,
                    in0=skrep,
                    in1=s_sb[:, 0:H, None].to_broadcast([HD, H, HD]),
                    op=mybir.AluOpType.mult,
                )

            # softmax per head-group (no max subtraction: |attn| <= temp),
            # T1_h = attn_h^T @ Wproj_h follows each group on PE
            t1_ctx = ExitStack()
            t1_pool = t1_ctx.enter_context(tc.tile_pool(name="t1p", bufs=1))
            t1_sb = t1_pool.tile([HD, H, C], F32R)
            with tc.tile_pool(name="t1ps", bufs=2, space="PSUM") as t1ps:
                for g, (h0, nh) in enumerate(((0, 5), (5, 3))):
                    bank = at1 if g == 0 else at2
                    ga = atsb[:, h0 : h0 + nh, :]
                    nc.vector.tensor_copy(
                        out=ga.rearrange("p h e -> p (h e)"), in_=bank
                    )
                    nc.vector.tensor_tensor(
                        out=ga, in0=ga, in1=skrep[:, h0 : h0 + nh, :],
                        op=mybir.AluOpType.mult,
                    )
                    nc.scalar.activation(
                        out=ga, in_=ga,
                        func=mybir.ActivationFunctionType.Exp,
                        bias=0.0, scale=1.0,
                    )
                    rsum = fsb.tile([HD, H], F32, tag="rsum")
                    nc.vector.tensor_reduce(
                        out=rsum[:, 0:nh], in_=ga, axis=mybir.AxisListType.X,
                        op=mybir.AluOpType.add,
                    )
                    nc.vector.reciprocal(out=rsum[:, 0:nh], in_=rsum[:, 0:nh])
                    nc.vector.tensor_tensor(
                        out=ga, in0=ga,
                        in1=rsum[:, 0:nh, None].to_broadcast([HD, nh, HD]),
                        op=mybir.AluOpType.mult,
                    )
                    for h in range(h0, h0 + nh):
                        t1p = t1ps.tile([HD, C], F32, tag="t1p")
                        lh = atsb[:, h, :]
                        nc.tensor.matmul(
                            t1p[:, 0:512], lhsT=lh, rhs=wpe_sb[:, h, 0:512],
                            start=True, stop=True,
                        )
                        nc.tensor.matmul(
                            t1p[:, 512:C], lhsT=lh, rhs=wpe_sb[:, h, 512:C],
                            start=True, stop=True,
                        )
                        nc.vector.tensor_copy(out=t1_sb[:, h, :], in_=t1p)
        if dbg:
            nc.scalar.dma_start(out=dbg_at[:, :, :], in_=atsb.bitcast(F32))
            nc.scalar.dma_start(out=dbg_t1[:, :, :], in_=t1_sb.bitcast(F32))
        at_ctx.close()

        # Wbig = sum_h Wv_h @ T1_h
        with tc.tile_pool(name="wbps", bufs=2, space="PSUM") as wbps:
            for m in range(CB):
                wbp = wbps.tile([P, C], F32, tag="wbp")
                for h in range(H):
                    lh = wvt_sb[:, h, m * P : (m + 1) * P]
                    nc.tensor.matmul(
                        wbp[:, 0:512], lhsT=lh, rhs=t1_sb[:, h, 0:512],
                        start=(h == 0), stop=(h == H - 1),
                        skip_group_check=True,
                    )
                    nc.tensor.matmul(
                        wbp[:, 512:C], lhsT=lh, rhs=t1_sb[:, h, 512:C],
                        start=(h == 0), stop=(h == H - 1),
                        skip_group_check=True,
                    )
                nc.vector.tensor_copy(out=wbig_sb[:, m, :], in_=wbp)
        if dbg:
            nc.scalar.dma_start(out=dbg_wb[:, :, :], in_=wbig_sb)
        t1_ctx.close()
        fs_ctx.close()
        w2_ctx.close()

        # ---------------- pass 2: y^T = Wbig^T @ x^T + b (bf16) -------------
        with tc.tile_pool(name="yps", bufs=8, space="PSUM") as yps:
            for n in range(8):
                nsl = slice(n * 512, (n + 1) * 512)
                xtc = cpool.tile([P, CB, 512], BF16, tag="xtc")
                nc.sync.dma_start(
                    out=xtc,
                    in_=xt[:, nsl].rearrange("(cb p) m -> p cb m", p=P),
                )
                for co in range(CB):
                    ypb = yps.tile([P, 512], F32, tag="ypb")
                    for ci in range(CB):
                        nc.tensor.matmul(
                            ypb,
                            lhsT=wbig_sb[:, ci, co * P : (co + 1) * P],
                            rhs=xtc[:, ci, :],
                            start=(ci == 0),
                            stop=(ci == CB - 1),
                        )
                    ysb = ypool.tile([P, 512], BF16, tag="ysb")
                    nc.vector.tensor_tensor(
                        out=ysb,
                        in0=ypb,
                        in1=bias_sb[:, co : co + 1].to_broadcast([P, 512]),
                        op=mybir.AluOpType.add,
                    )
                    nc.scalar.dma_start(
                        out=yt[co * P : (co + 1) * P, nsl], in_=ysb
                    )
        c_ctx.close()

    nc.compile()
    return nc


def prep_inputs(x, Wqkv, temperature, Wproj, bproj):
    import ml_dtypes

    B = x.shape[0]
    wqkv = np.asarray(Wqkv, dtype=np.float32)
    w2 = np.ascontiguousarray(wqkv[:, :C2])
    wvt = np.ascontiguousarray(
        wqkv[:, C2:].T.reshape(H, HD, C).transpose(1, 0, 2)
    )
    wpe = np.ascontiguousarray(
        np.asarray(Wproj, dtype=np.float32).reshape(H, HD, C).transpose(1, 0, 2)
    )
    temp = np.ascontiguousarray(np.asarray(temperature, dtype=np.float32).reshape(H))
    biasE = np.ascontiguousarray(
        np.asarray(bproj, dtype=np.float32).reshape(CB, P).T
    )
    xf = np.asarray(x, dtype=np.float32)
    in_maps = [
        {
            "x": np.ascontiguousarray(xf[b].astype(ml_dtypes.bfloat16)),
            "xt": np.ascontiguousarray(xf[b].T.astype(ml_dtypes.bfloat16)),
            "w2": w2,
            "wvt": wvt,
            "wpe": wpe,
            "temp": temp,
            "biasE": biasE,
        }
        for b in range(B)
    ]
    return in_maps


def kernel(x, Wqkv, temperature, Wproj, bproj):
    from concourse.bass_utils import run_bass_kernel_spmd

    B = x.shape[0]
    key = "nc"
    if key not in _CACHE:
        _CACHE[key] = _build()
    nc = _CACHE[key]

    in_maps = prep_inputs(x, Wqkv, temperature, Wproj, bproj)
    res = run_bass_kernel_spmd(nc, in_maps, core_ids=list(range(B)))
    out = np.stack(
        [res.results[b]["yt"].T.astype(np.float32) for b in range(B)], axis=0
    )
    return np.ascontiguousarray(out)


if __name__ == "__main__":
    rng = np.random.default_rng(0)
    inputs = {
        "x": rng.standard_normal((8, N, C), dtype=np.float32),
        "Wqkv": (rng.standard_normal((C, NC3)) / np.sqrt(C)).astype(np.float32),
        "temperature": np.ones((H, 1, 1), dtype=np.float32),
        "Wproj": (rng.standard_normal((C, C)) / np.sqrt(C)).astype(np.float32),
        "bproj": (rng.standard_normal(C) * 0.01).astype(np.float32),
    }
    out = kernel(**inputs)
    print(out.shape, out.dtype)
